# revision 71
# baseline (speedup 1.0000x reference)
"""Multi-head attention (B=2, S=2048, D=1024, H=16) on 8 Trainium2 NeuronCores.

Sharding: head-parallel attention (2 heads/core, both batches); 4 chunked
AllToAlls (one per 1024-token attention block, interleaved 128-token slots)
redistribute attention outputs so each core runs the output projection for
4x128 tokens per round, overlapped with later attention blocks.

Per-core pipeline (core c, heads hA=2c, hB=2c+1), all matmuls bf16:
  - QKV of quarter 0 runs as a dense serial head; QKV of quarters 1-3
    (projection groups split into 4-MM atoms) + V transposes overlay the
    attention blocks' j-loops, Wo rounds 0-2 overlay blocks 2-3, all
    interleaved into the in-order PE queue to keep the PE dense.
  - scores^T per j (128 keys): two heads row-packed (tile_position (0,0)
    and (64,0)) run concurrently; per-head [128,1024] exp on ScalarE with
    fused 1/8 scale, pipelined so one head's scores hide under the other
    head's exp (ps tiles single-buffered per head).
  - AV/den lag two j's behind the scores so the next j's score matmuls sit
    directly behind this j's in the PE queue.  AV: col-packed M=64 (head A
    -> psum partitions 0-63 at (0,0), head B -> 64-127 at (0,64)) into one
    shared [128,1024] accumulator; softmax denominators via a 4-way
    col-packed M=1 ones-matmul wave into one [128,512] bank (partitions
    0/32/64/96).
  - normalize: den broadcast across partitions via K=1 ones-matmuls (DVE
    reciprocal and gpsimd partition_broadcast are broken for APs at base
    partition != 0), reciprocal at base 0, DVE mul -> bf16; 8 slots ->
    DRAM -> per-block AllToAll round -> Wo matmuls + bias per round.
  - tail: dummy matmuls keep the PE clock warm (HAM) while the last
    AllToAll round is in flight.
PSUM in attention: scores 2x2 banks + av 2 + den 1 + overlay 1 = 8.
"""

import numpy as np

B, S, D, H, HD = 2, 2048, 1024, 16, 64
NCORES = 8
BT = B * S
SCALE = 1.0 / 8.0
BLOCKS = [(0, 0), (0, 1), (1, 0), (1, 1)]

_CACHE = {}


def _build():
    import concourse.bacc as bacc
    import concourse.tile as tile
    import concourse.mybir as mybir

    F32 = mybir.dt.float32
    BF16 = mybir.dt.bfloat16
    EXP = mybir.ActivationFunctionType.Exp

    nc = bacc.Bacc("TRN2", target_bir_lowering=False, debug=False,
                   num_devices=NCORES)

    # ---- I/O ------------------------------------------------------------
    # xt: per (quarter, ch-half): [128 D-partitions, 8 kblocks x 512 tokens]
    # partition-major (8KB contiguous per partition) for full-rate DMA.
    xt = nc.dram_tensor("xt", [4, 2, 128, 4096], BF16, kind="ExternalInput")
    # proj-major: [which(K,Q,V), 128, 8 kblocks, 128] so the K columns can
    # land first and unblock the serial head early.
    wqkv = nc.dram_tensor("wqkv", [3, 128, 8 * 128], BF16,
                          kind="ExternalInput")
    bq = nc.dram_tensor("bq", [128, 1], F32, kind="ExternalInput")
    bk = nc.dram_tensor("bk", [128, 1], F32, kind="ExternalInput")
    bv = nc.dram_tensor("bv", [128, 1], F32, kind="ExternalInput")
    wo = nc.dram_tensor("wo", [128, 8 * D], BF16, kind="ExternalInput")
    bo = nc.dram_tensor("bo", [1, D], BF16, kind="ExternalInput")
    eye = nc.dram_tensor("eye", [128, 128], BF16, kind="ExternalInput")
    ones64 = nc.dram_tensor("ones64", [128, 64], BF16, kind="ExternalInput")
    out = nc.dram_tensor("out", [512, D], F32, kind="ExternalOutput")

    KB = D // 128     # 8 contraction blocks
    groups = [list(range(NCORES))]

    with tile.TileContext(nc) as tc:
        from contextlib import ExitStack
        with ExitStack() as ctx:
            persist = ctx.enter_context(tc.tile_pool(name="persist", bufs=1))
            dram = ctx.enter_context(
                tc.tile_pool(name="dram", bufs=1, space="DRAM"))

            # ---- constant + input loads, split across the 2 HWDGE rings.
            # Sync ring: x chunks (1 MB each, partition-contiguous).
            # Scalar ring (idle until the first exp): weights + consts.
            wproj = []
            for which in range(3):
                t = persist.tile([128, KB, 128], BF16, tag=f"wproj{which}",
                                 name=f"wproj{which}")
                wproj.append(t)
            for which in (1, 0, 2):   # K columns first
                nc.scalar.dma_start(
                    wproj[which][:].rearrange("p k c -> p (k c)"),
                    wqkv[which])

            def wslice(which, k):
                return wproj[which][:, k]
            bq_sb = persist.tile([128, 1], F32, tag="bq")
            bk_sb = persist.tile([128, 1], F32, tag="bk")
            bv_sb = persist.tile([128, 1], F32, tag="bv")
            nc.scalar.dma_start(bq_sb[:], bq[:])
            nc.scalar.dma_start(bk_sb[:], bk[:])
            nc.scalar.dma_start(bv_sb[:], bv[:])
            eye_sb = persist.tile([128, 128], BF16, tag="eye")
            nc.scalar.dma_start(eye_sb[:], eye[:])
            ones_sb = persist.tile([128, 64], BF16, tag="ones64")
            nc.scalar.dma_start(ones_sb[:], ones64[:])

            # persistent activations
            qT = [persist.tile([128, S], BF16, tag=f"qT{b}", name=f"qT{b}")
                  for b in range(B)]
            kT = [persist.tile([128, S], BF16, tag=f"kT{b}", name=f"kT{b}")
                  for b in range(B)]

            # V in [128 keys, 2 heads, 64 dims] layout per 128-token block
            vv = [persist.tile([128, 2, HD], BF16, tag=f"v{tb}",
                               name=f"v{tb}")
                  for tb in range(BT // 128)]

            vt_pool = ctx.enter_context(tc.tile_pool(name="vtmp", bufs=4))

            xbig = {}

            def load_quarter(q):
                for ch in range(2):
                    t = persist.tile([128, 4096], BF16, tag=f"x{q}_{ch}",
                                     name=f"x{q}_{ch}")
                    if q == 0 and ch == 0:
                        # split so the serial head's first MMs (k=0..3)
                        # unblock after half the transfer
                        nc.sync.dma_start(t[:, 0:2048], xt[q, ch, :, 0:2048])
                        nc.sync.dma_start(t[:, 2048:4096],
                                          xt[q, ch, :, 2048:4096])
                    else:
                        nc.sync.dma_start(t[:], xt[q, ch])
                    xbig[(q, ch)] = t

            def emit_proj_group(pool, q, ch, which):
                """One accumulation group: 8 MMs. which: 0=Q, 1=K, 2=V."""
                acc = pool.tile([128, 512], F32, tag="ov", name="ovacc")
                xb = xbig[(q, ch)]
                for k in range(KB):
                    nc.tensor.matmul(
                        acc[:], wslice(which, k),
                        xb[:, k * 512:(k + 1) * 512],
                        start=(k == 0), stop=(k == KB - 1))
                return acc

            def emit_proj_drain(acc, q, ch, which):
                """Bias-add PSUM->SBUF. Returns vt tile for V, else None."""
                b = q // 2
                lo = (q % 2) * 1024 + ch * 512
                if which == 0:
                    nc.vector.tensor_scalar_add(
                        qT[b][:, lo:lo + 512], acc[:], bq_sb[:])
                    return None
                if which == 1:
                    nc.vector.tensor_scalar_add(
                        kT[b][:, lo:lo + 512], acc[:], bk_sb[:])
                    return None
                vt = vt_pool.tile([128, 512], BF16, tag="vt")
                nc.vector.tensor_scalar_add(vt[:], acc[:], bv_sb[:])
                return vt

            def emit_v_transpose(pool, vt, q, ch, blk):
                """PE-transpose one 128-token block of vt into vv."""
                tb = q * 8 + ch * 4 + blk
                ovt = pool.tile([128, 512], F32, tag="ov", name="ovacc")
                pv = ovt[:].bitcast(BF16)
                nc.tensor.transpose(
                    pv[:, 0:128], vt[:, blk * 128:(blk + 1) * 128],
                    eye_sb[:])
                nc.vector.tensor_copy(
                    vv[tb][:], pv[:, 0:128].rearrange(
                        "p (h d) -> p h d", h=2))

            # ---- serial head: K-c0, Q-c0, Q-c1, V-c0 of quarter 0 --------
            load_quarter(0)
            load_quarter(1)
            load_quarter(2)
            load_quarter(3)
            wo_big = persist.tile([128, NCORES, D], BF16, tag="wo",
                                  name="wo_big")
            nc.scalar.dma_start(wo_big[:].rearrange("p s c -> p (s c)"),
                                wo[:])
            wo_sb = [wo_big[:, r] for r in range(NCORES)]
            bo_bc = persist.tile([128, D], BF16, tag="bo_bc")
            nc.scalar.dma_start(bo_bc[:], bo[:].to_broadcast((128, D)))
            # serial head: K-c0, Q-c0, Q-c1, V-c0 (+transposes) of quarter 0
            with tc.tile_pool(name="p0_ps", bufs=3, space="PSUM") as p0:
                for ch, which in ((0, 1), (0, 0), (1, 0), (0, 2)):
                    acc = emit_proj_group(p0, 0, ch, which)
                    vt = emit_proj_drain(acc, 0, ch, which)
                    if vt is not None:
                        for blk in range(4):
                            emit_v_transpose(p0, vt, 0, ch, blk)

            # ---- attention-phase pools ----------------------------------
            ov_pool = ctx.enter_context(
                tc.tile_pool(name="ov_ps", bufs=2, space="PSUM"))
            ps_pool = ctx.enter_context(
                tc.tile_pool(name="sc_ps", bufs=1, space="PSUM"))
            av_pool = ctx.enter_context(
                tc.tile_pool(name="av_ps", bufs=1, space="PSUM"))
            pt_pool = ctx.enter_context(tc.tile_pool(name="pt", bufs=6))
            den_pool = ctx.enter_context(tc.tile_pool(name="den", bufs=2))
            an_pool = ctx.enter_context(tc.tile_pool(name="an", bufs=2))
            rc_pool = ctx.enter_context(tc.tile_pool(name="rc", bufs=2))
            ko_pool = ctx.enter_context(tc.tile_pool(name="ko", bufs=2))
            ot_pool = ctx.enter_context(tc.tile_pool(name="osb", bufs=2))

            # ---- overlay worklists for the attention blocks -------------
            # Atoms of <=4 matmuls with a PE-cost estimate (ns); paced by
            # cumulative cost across the 16 j-slots of each block, with
            # leftover spilling to the next block's front.
            vt_hold = []  # V-proj drains, consumed by transposes

            def ov_qkv(q, ch, which):
                hold = {}
                xb = xbig[(q, ch)]

                def atom(k0):
                    def emit():
                        if k0 == 0:
                            hold[0] = ov_pool.tile([128, 512], F32,
                                                   tag="ov", name="ovacc")
                        acc = hold[0]
                        for k in (k0, k0 + 1):
                            nc.tensor.matmul(
                                acc[:], wslice(which, k),
                                xb[:, k * 512:(k + 1) * 512],
                                start=(k == 0), stop=(k == KB - 1))
                        if k0 == KB - 2:
                            vt = emit_proj_drain(acc, q, ch, which)
                            if vt is not None:
                                vt_hold.append((vt, q, ch))
                    return (530, emit)
                return [atom(k0) for k0 in range(0, KB, 2)]

            def ov_vtrans(idx, blk):
                def emit():
                    vt, q, ch = vt_hold[idx]
                    emit_v_transpose(ov_pool, vt, q, ch, blk)
                return (160, emit)

            ov_work = {0: [], 1: [], 2: [], 3: []}
            vt_idx = [0]

            def add_qkv(dst, q, ch, which):
                ov_work[dst].extend(ov_qkv(q, ch, which))
                if which == 2:
                    idx = vt_idx[0]
                    vt_idx[0] += 1
                    for blk in range(4):
                        ov_work[dst].append(ov_vtrans(idx, blk))

            # block 0 front: V-c0 (vv[0..3] by AV(0) at iter 3), K-c1 of
            # quarter 0 (scores j=4), V-c1, then quarter 1 K/V just-in-
            # time and Q-q1 (needed by block 1's carried scores at
            # block-0 end); block 1: quarter 2; block 2: K/V-q3 then
            # Q-q3 (carried scores of block 3 need them by block-2 end);
            # block 3: Wo r0-r2.
            add_qkv(0, 0, 1, 1)
            add_qkv(0, 0, 1, 2)
            add_qkv(0, 1, 0, 1)
            add_qkv(0, 1, 0, 2)
            add_qkv(0, 1, 0, 0)
            add_qkv(0, 1, 1, 1)
            add_qkv(0, 1, 1, 2)
            add_qkv(0, 1, 1, 0)
            for ch in range(2):
                for which in (1, 2, 0):
                    add_qkv(1, 2, ch, which)
            for ch in range(2):
                for which in (1, 2):
                    add_qkv(2, 3, ch, which)
            for ch in range(2):
                add_qkv(2, 3, ch, 0)

            # ---- exchange + Wo machinery --------------------------------
            a2a_in = [dram.tile([NCORES, 128, 128], BF16,
                                name=f"a2a_in{r}") for r in range(3)]
            a2a_out = [dram.tile([NCORES, 128, 128], BF16,
                                 name=f"a2a_out{r}") for r in range(3)]
            # block 3 exchanges in two query-halves so Wo can start on the
            # first half while the second is still in flight
            a2a3h_in = [dram.tile([NCORES, 128, 64], BF16,
                                  name=f"a2a3h_in{r}") for r in range(2)]
            a2a3h_out = [dram.tile([NCORES, 128, 64], BF16,
                                   name=f"a2a3h_out{r}") for r in range(2)]

            def ov_wo_round(rnd):
                """Wo chunk closures for one received round."""
                chunks = []
                ko_hold = {}

                def emit_load():
                    t = ko_pool.tile([128, NCORES, 128], BF16, tag="ko")
                    nc.sync.dma_start(
                        t[:], a2a_out[rnd][:].rearrange("s p f -> p s f"))
                    ko_hold[0] = t
                chunks.append((80, emit_load))

                def half(nh):
                    hold = {}

                    def atom(s0):
                        def emit():
                            if s0 == 0:
                                hold[0] = ov_pool.tile([128, 512], F32,
                                                       tag="ov",
                                                       name="ovacc")
                            acc = hold[0]
                            ko = ko_hold[0]
                            for s in (s0, s0 + 1):
                                nc.tensor.matmul(
                                    acc[:], ko[:, s, :],
                                    wo_sb[s][:, nh * 512:(nh + 1) * 512],
                                    start=(s == 0),
                                    stop=(s == NCORES - 1))
                            if s0 == NCORES - 2:
                                ot = ot_pool.tile([128, 512], F32,
                                                  tag="ot")
                                nc.vector.tensor_add(
                                    ot[:], acc[:],
                                    bo_bc[:, nh * 512:(nh + 1) * 512])
                                nc.sync.dma_start(
                                    out[rnd * 128:(rnd + 1) * 128,
                                        nh * 512:(nh + 1) * 512], ot[:])
                        return (530, emit)
                    return [atom(s0) for s0 in range(0, NCORES, 2)]
                chunks.extend(half(0))
                chunks.extend(half(1))
                return chunks

            # Wo rounds 0-2 overlay block 3 (their a2a rounds are long done).
            ov_work[3].extend(ov_wo_round(0))
            ov_work[3].extend(ov_wo_round(1))
            ov_work[3].extend(ov_wo_round(2))

            def ov_filler():
                def emit():
                    dmy = ov_pool.tile([128, 512], F32, tag="ov",
                                       name="ovacc")
                    nc.tensor.matmul(dmy[:], wslice(0, 0),
                                     qT[0][:, 0:512], start=True, stop=True)
                return (270, emit)

            # ---- attention blocks ---------------------------------------
            def emit_scores(b, ich, j):
                """4 score MMs (row-packed head pairs) + per-head FD=1024
                exps into one shared [128, 2048] bf16 tile (A | B)."""
                qlo = ich * 1024
                klo = j * 128
                pth = pt_pool.tile([128, 2048], BF16, tag="pt", name="pth")
                for h in range(2):
                    psh = ps_pool.tile([128, 1024], F32, tag=f"ps{h}",
                                       name=f"ps{h}")
                    for sub in range(2):
                        nc.tensor.matmul(
                            psh[:, sub * 512:(sub + 1) * 512],
                            kT[b][h * 64:(h + 1) * 64, klo:klo + 128],
                            qT[b][h * 64:(h + 1) * 64,
                                  qlo + sub * 512:qlo + (sub + 1) * 512],
                            start=True, stop=True,
                            tile_position=(h * 64, 0))
                    nc.scalar.activation(
                        pth[:, h * 1024:(h + 1) * 1024], psh[:],
                        EXP, scale=SCALE)
                return pth

            # one global overlay queue with per-block cumulative targets
            ov_flat = []
            ov_tgt = []
            for blk in range(4):
                ov_flat.extend(ov_work[blk])
                ov_tgt.append(sum(c for c, _ in ov_flat))
            ov_ptr = [0]
            ov_done = [0]

            def pace(target):
                while ov_ptr[0] < len(ov_flat) and ov_done[0] < target:
                    c, fn = ov_flat[ov_ptr[0]]
                    fn()
                    ov_done[0] += c
                    ov_ptr[0] += 1

            hist = {}
            for blk, (b, ich) in enumerate(BLOCKS):
                av = av_pool.tile([128, 1024], F32, tag="av")
                tgt0 = ov_tgt[blk - 1] if blk else 0
                span = ov_tgt[blk] - tgt0
                acc_d = [None]

                def den_add(j, hist=hist, acc_d=acc_d):
                    """acc_d accumulates exp sums over j=0..14."""
                    t = den_pool.tile([128, 2048], BF16, tag="den")
                    if j == 1:
                        nc.vector.tensor_add(t[:], hist[0][:], hist[1][:])
                    else:
                        nc.vector.tensor_add(t[:], acc_d[0][:], hist[j][:])
                    acc_d[0] = t

                def emit_av(j, pth, av=av, b=b):
                    tb = b * 16 + j
                    for sub in range(2):
                        lo, hi = sub * 512, (sub + 1) * 512
                        nc.tensor.matmul(
                            av[0:64, lo:hi], vv[tb][:, 0, :],
                            pth[:, lo:hi], start=(j == 0), stop=(j == 15),
                            tile_position=(0, 0))
                        nc.tensor.matmul(
                            av[64:128, lo:hi], vv[tb][:, 1, :],
                            pth[:, 1024 + lo:1024 + hi],
                            start=(j == 0), stop=(j == 15),
                            tile_position=(0, 64))

                j0 = 2 if blk else 0
                for j in range(j0, 16):
                    hist[j] = emit_scores(b, ich, j)
                    if j >= 2:
                        den_add(j - 1)
                        emit_av(j - 2, hist.pop(j - 2))
                    pace(tgt0 + (span * (j - j0 + 1)) // (16 - j0))
                # block tail: carry next block's first two score/exp pairs
                # so the ACT stream never breaks at the boundary.
                if blk < 3:
                    nb, nich = BLOCKS[blk + 1]
                    nxt0 = emit_scores(nb, nich, 0)
                emit_av(14, hist.pop(14))
                if blk < 3:
                    nxt1 = emit_scores(nb, nich, 1)
                pth15 = hist.pop(15)
                emit_av(15, pth15)

                # normalize: per (head, sub) an all-ones K=128/M=64 matmul
                # pair column-sums acc_d (exps j=0..14) then pth15, giving
                # the denominator replicated across 64 partitions with no
                # final DVE add on the critical path.
                an = an_pool.tile([128, 1024], BF16, tag="an")
                accT = acc_d[0]
                for sub in range(2):
                    ovn = ov_pool.tile([128, 512], F32, tag="ov",
                                       name="ovacc")
                    for h in range(2):
                        nc.tensor.matmul(
                            ovn[h * 64:(h + 1) * 64, :],
                            ones_sb[:, 0:64],
                            accT[:, h * 1024 + sub * 512:
                                 h * 1024 + (sub + 1) * 512],
                            start=True, stop=False,
                            tile_position=(0, h * 64))
                    for h in range(2):
                        nc.tensor.matmul(
                            ovn[h * 64:(h + 1) * 64, :],
                            ones_sb[:, 0:64],
                            pth15[:, h * 1024 + sub * 512:
                                  h * 1024 + (sub + 1) * 512],
                            start=False, stop=True,
                            tile_position=(0, h * 64))
                    rec = rc_pool.tile([128, 512], F32, tag="rec")
                    nc.vector.reciprocal_approx_fast(rec[:], ovn[:])
                    nc.vector.tensor_mul(
                        an[:, sub * 512:(sub + 1) * 512],
                        av[:, sub * 512:(sub + 1) * 512], rec[:])
                    if blk < 3:
                        nc.sync.dma_start(
                            a2a_in[blk][sub * 4:(sub + 1) * 4].rearrange(
                                "s p f -> p s f"),
                            an[:, sub * 512:(sub + 1) * 512].rearrange(
                                "p (s f) -> p s f", s=4))
                if blk < 3:
                    nc.gpsimd.collective_compute(
                        "AllToAll", mybir.AluOpType.bypass,
                        ins=[a2a_in[blk][:]], outs=[a2a_out[blk][:]],
                        replica_groups=groups)
                    hist = {0: nxt0, 1: nxt1}
                else:
                    anv = an[:].rearrange("p (s f) -> p s f", s=8)
                    for hf in range(2):
                        nc.sync.dma_start(
                            a2a3h_in[hf][:].rearrange("s p f -> p s f"),
                            anv[:, :, hf * 64:(hf + 1) * 64])
                        nc.gpsimd.collective_compute(
                            "AllToAll", mybir.AluOpType.bypass,
                            ins=[a2a3h_in[hf][:]],
                            outs=[a2a3h_out[hf][:]],
                            replica_groups=groups)

            # ---- tail: keep the PE warm while the two half-exchanges of
            # round 3 are in flight; Wo for half a overlaps half b's a2a.
            # Each half: M=64 queries with the two output-dim halves
            # col-packed into one [128, 512] psum tile.
            koh = {}
            for hf in range(2):
                t = ko_pool.tile([128, NCORES, 64], BF16, tag="koh",
                                 name="koh")
                nc.sync.dma_start(
                    t[:], a2a3h_out[hf][:].rearrange("s p f -> p s f"))
                koh[hf] = t

            def wo_half(hf):
                acc = ov_pool.tile([128, 512], F32, tag="ov", name="ovacc")
                for s in range(NCORES):
                    for nh in range(2):
                        nc.tensor.matmul(
                            acc[nh * 64:(nh + 1) * 64, :], koh[hf][:, s, :],
                            wo_sb[s][:, nh * 512:(nh + 1) * 512],
                            start=(s == 0), stop=(s == NCORES - 1),
                            tile_position=(0, nh * 64))
                ot = ot_pool.tile([128, 512], F32, tag="ot")
                for nh in range(2):
                    nc.vector.tensor_add(
                        ot[nh * 64:(nh + 1) * 64, :],
                        acc[nh * 64:(nh + 1) * 64, :],
                        bo_bc[nh * 64:(nh + 1) * 64,
                              nh * 512:(nh + 1) * 512])
                    nc.sync.dma_start(
                        out[384 + hf * 64:448 + hf * 64,
                            nh * 512:(nh + 1) * 512],
                        ot[nh * 64:(nh + 1) * 64, :])

            def dummies(n):
                for _ in range(n):
                    dmy = ov_pool.tile([128, 512], F32, tag="ov",
                                       name="ovacc")
                    nc.tensor.matmul(dmy[:], wslice(0, 0),
                                     qT[0][:, 0:512], start=True, stop=True)
            dummies(40)
            wo_half(0)
            dummies(15)
            wo_half(1)

    nc.compile()
    return nc


def _get_nc():
    if "nc" not in _CACHE:
        _CACHE["nc"] = _build()
    return _CACHE["nc"]


def _make_in_maps(hidden_states, Wq, bq, Wk, bk, Wv, bv, Wo, bo):
    import ml_dtypes
    bf16 = ml_dtypes.bfloat16
    hs = np.ascontiguousarray(np.asarray(hidden_states, dtype=np.float32))
    xT = hs.reshape(BT, D).T.astype(bf16)
    # pre-tiled: [quarter, ch-half, 128 partitions, kblock*512] — each
    # partition's 8 KB is contiguous in DRAM for full-rate DMA.
    xt = np.ascontiguousarray(
        xT.reshape(8, 128, 4, 2, 512).transpose(2, 3, 1, 0, 4).reshape(
            4, 2, 128, 4096))
    eye = np.eye(128, dtype=bf16)
    ones64 = np.ones((128, 64), dtype=bf16)
    Wq = np.asarray(Wq, np.float32).astype(bf16)
    Wk = np.asarray(Wk, np.float32).astype(bf16)
    Wv = np.asarray(Wv, np.float32).astype(bf16)
    Wo = np.asarray(Wo, np.float32).astype(bf16)
    # wo: [128 partitions, 8 kblocks * 1024] partition-major
    Wo_t = np.ascontiguousarray(
        Wo.reshape(8, 128, D).transpose(1, 0, 2).reshape(128, 8 * D))
    bq = np.asarray(bq, np.float32); bk = np.asarray(bk, np.float32)
    bv = np.asarray(bv, np.float32); bo = np.asarray(bo, np.float32)
    in_maps = []
    for c in range(NCORES):
        sl = slice(2 * c * HD, (2 * c + 2) * HD)
        # [which(Q,K,V), 128 partitions, 8 kblocks * 128] proj-major
        wqkv_t = np.ascontiguousarray(
            np.stack([Wq[:, sl], Wk[:, sl], Wv[:, sl]]).reshape(
                3, 8, 128, 128).transpose(0, 2, 1, 3).reshape(
                3, 128, 8 * 128))
        in_maps.append({
            "xt": xt,
            "wqkv": wqkv_t,
            "bq": np.ascontiguousarray(bq[sl].reshape(128, 1)),
            "bk": np.ascontiguousarray(bk[sl].reshape(128, 1)),
            "bv": np.ascontiguousarray(bv[sl].reshape(128, 1)),
            "wo": Wo_t,
            "bo": np.ascontiguousarray(bo.reshape(1, D)).astype(bf16),
            "eye": eye,
            "ones64": ones64,
        })
    return in_maps


def run(trace=False, tmpdir=None, **inputs):
    from concourse.bass_utils import run_bass_kernel_spmd
    nc = _get_nc()
    in_maps = _make_in_maps(**inputs)
    res = run_bass_kernel_spmd(nc, in_maps, list(range(NCORES)), trace=trace,
                               tmpdir=tmpdir)
    full = np.empty((B, S, D), dtype=np.float32)
    for c in range(NCORES):
        o = res.results[c]["out"]
        for rnd, (b, ich) in enumerate(BLOCKS):
            t0 = ich * 1024 + c * 128
            full[b, t0:t0 + 128, :] = o[rnd * 128:(rnd + 1) * 128]
    return full, res


def kernel(**inputs) -> np.ndarray:
    out, _ = run(trace=False, **inputs)
    return out

